# revision 1
# baseline (speedup 1.0000x reference)
"""Trainium2 Bass kernel for a pre-norm transformer encoder layer (SwiGLU FFN).

Shapes (hardcoded): x [2, 2048, 768], mask [2, 2048, 2048] int32,
wq/wk/wv/wo [768, 768], w1/w3 [3072, 768], w2 [768, 3072], g_attn/g_ffn [768].

Sharding: 8 cores = 2 batch x 4 query-slices of 512 tokens. Each core
computes K/V for its full batch element (replicated within the group of 4)
and attention + FFN for its own 512 tokens. No collectives.

On-device layout is feature-major ("transposed"): activations [D, tokens].
All matmuls run in bf16 with fp32 PSUM accumulation.
"""
import os
import sys

for _p in ("/opt/trn_rl_repo", "/root/.axon_site/_ro/trn_rl_repo"):
    if os.path.isdir(_p) and _p not in sys.path:
        sys.path.append(_p)

import numpy as np
import ml_dtypes

import concourse.bacc as bacc
import concourse.tile as tile
from concourse import mybir

F32 = mybir.dt.float32
BF16 = mybir.dt.bfloat16
AF = mybir.ActivationFunctionType

B, S, D, H = 2, 2048, 768, 12
DK = D // H            # 64
F = 4 * D              # 3072
T = 512                # local query tokens per core
NCH = D // 128         # 6 feature chunks
NFC = F // 128         # 24 FFN chunks
NKT = S // 128         # 16 key tiles
NQT = S // T           # 4 query slices per batch element
EPS = 1e-5


def build_nc():
    nc = bacc.Bacc("TRN2", target_bir_lowering=False, debug=False, num_devices=8)

    xT = nc.dram_tensor("xT", [NCH, 128, S], F32, kind="ExternalInput").ap()
    maskT = nc.dram_tensor("maskT", [128, NKT * T], BF16, kind="ExternalInput").ap()
    wqT = nc.dram_tensor("wqT", [NCH, 128, D], BF16, kind="ExternalInput").ap()
    wkT = nc.dram_tensor("wkT", [NCH, 128, D], BF16, kind="ExternalInput").ap()
    wvT = nc.dram_tensor("wvT", [NCH, 128, D], BF16, kind="ExternalInput").ap()
    woT = nc.dram_tensor("woT", [H, DK, D], BF16, kind="ExternalInput").ap()
    w1T = nc.dram_tensor("w1T", [NFC, 128, D], BF16, kind="ExternalInput").ap()
    w3T = nc.dram_tensor("w3T", [NFC, 128, D], BF16, kind="ExternalInput").ap()
    w2T = nc.dram_tensor("w2T", [NCH, 128, F], BF16, kind="ExternalInput").ap()
    ones16 = nc.dram_tensor("ones16", [128, 128], BF16, kind="ExternalInput").ap()

    outT = nc.dram_tensor("outT", [NCH, 128, T], F32, kind="ExternalOutput").ap()
    warm_out = nc.dram_tensor("warm_out", [5, 128, T], BF16,
                              kind="ExternalOutput").ap()

    with tile.TileContext(nc) as tc:
        with tc.tile_pool(name="glob", bufs=1) as Pg:
            ones16_t = Pg.tile([128, 128], BF16, name="ones16_t")
            nc.sync.dma_start(ones16_t[:], ones16)
            def warm_burst(idx, psum_pool, tag, sbuf_pool, rhs, pbufs, n=12,
                           dve=False):
                """Dense full-width matmul burst to trip the PE HAM to 2.4GHz."""
                wp = psum_pool.tile([128, T], F32, tag=tag, bufs=pbufs,
                                    name=f"wrm_ps{idx}")
                for i in range(n):
                    nc.tensor.matmul(wp[:], ones16_t[:], rhs,
                                     start=(i == 0), stop=(i == n - 1))
                ws = sbuf_pool.tile([128, T], BF16, tag="wrm_sb",
                                    name=f"wrm_sb{idx}")
                if dve:
                    nc.vector.tensor_copy(ws[:], wp[:])
                else:
                    nc.scalar.copy(ws[:], wp[:])
                nc.sync.dma_start(warm_out[idx], ws[:])

            eps_t = Pg.tile([128, 1], F32, name="eps_t")
            nc.vector.memset(eps_t[:], EPS)
            xloc = [Pg.tile([128, T], F32, name=f"xloc{c}") for c in range(NCH)]
            hT = [Pg.tile([128, T], F32, name=f"hT{c}") for c in range(NCH)]
            attnT = [Pg.tile([DK, T], BF16, name=f"attnT{h}") for h in range(H)]
            warm_rhs = Pg.tile([128, T], BF16, name="warm_rhs")
            nc.sync.dma_start(warm_rhs[:], maskT[:, 0:T])

            with tc.tile_pool(name="attn", bufs=1) as Pa:
                KT = [Pa.tile([128, S], BF16, name=f"KT{c}") for c in range(NCH)]
                QT = [Pa.tile([128, T], BF16, name=f"QT{c}") for c in range(NCH)]
                VA = [Pa.tile([128, H * (DK + 1)], BF16, name=f"VA{t}")
                      for t in range(NKT)]
                maskT_t = Pa.tile([128, NKT * T], BF16, name="maskT_t")

                # ---------------- stage 1: rmsnorm + Q/K/V projections --------
                with (
                    tc.tile_pool(name="s1", bufs=1) as P1,
                    tc.tile_pool(name="ps1", bufs=1, space="PSUM") as PS1,
                ):
                    wq_t = [P1.tile([128, D], BF16, name=f"wq{c}") for c in range(NCH)]
                    wk_t = [P1.tile([128, D], BF16, name=f"wk{c}") for c in range(NCH)]
                    wv_t = [P1.tile([128, D], BF16, name=f"wv{c}") for c in range(NCH)]

                    for qt in range(NQT):
                        sl = slice(qt * T, (qt + 1) * T)
                        if qt > 0:
                            warm_burst(0, PS1, "ps_q", P1, warm_rhs[:], 1, n=12)
                        xq = [P1.tile([128, T], F32, name=f"xq{c}", tag=f"xq{c}")
                              for c in range(NCH)]
                        for c in range(NCH):
                            nc.sync.dma_start(xq[c][:], xT[c][:, sl])
                        if qt == 0:
                            for c in range(NCH):
                                nc.sync.dma_start(wq_t[c][:], wqT[c])
                                nc.sync.dma_start(wk_t[c][:], wkT[c])
                                nc.sync.dma_start(wv_t[c][:], wvT[c])
                        ps_ms = PS1.tile([128, T], F32, tag="ps_ms", name="ps_ms")
                        for c in range(NCH):
                            sq = P1.tile([128, T], BF16, tag="sq", bufs=2,
                                         name=f"sq{qt}_{c}")
                            nc.vector.tensor_mul(sq[:], xq[c][:], xq[c][:])
                            nc.tensor.matmul(ps_ms[:], ones16_t[:], sq[:],
                                             start=(c == 0), stop=(c == NCH - 1))
                        lntmp = P1.tile([128, T], F32, tag="lntmp", bufs=2,
                                        name=f"ln{qt}")
                        nc.scalar.activation(lntmp[:], ps_ms[:], AF.Ln,
                                             bias=eps_t[:], scale=1.0 / D)
                        rstd = P1.tile([128, T], F32, tag="rstd", bufs=2,
                                       name=f"rstd{qt}")
                        nc.scalar.activation(rstd[:], lntmp[:], AF.Exp, scale=-0.5)
                        xn = [P1.tile([128, T], BF16, name=f"xn{c}", tag=f"xn{c}",
                                      bufs=2) for c in range(NCH)]
                        for c in range(NCH):
                            nc.vector.tensor_mul(xn[c][:], xq[c][:], rstd[:])

                        local = (qt == QT_IDX)
                        if local:
                            for c in range(NCH):
                                nc.vector.tensor_copy(xloc[c][:], xq[c][:])
                        # K projection (and Q for the local slice)
                        for do in range(NCH):
                            ps_k = PS1.tile([128, T], F32, tag="ps_k", bufs=2,
                                            name=f"ps_k{qt}_{do}")
                            for c in range(NCH):
                                nc.tensor.matmul(
                                    ps_k[:], wk_t[c][:, do * 128:(do + 1) * 128],
                                    xn[c][:], start=(c == 0), stop=(c == NCH - 1))
                            nc.scalar.copy(KT[do][:, sl], ps_k[:])
                            if local:
                                ps_q = PS1.tile([128, T], F32, tag="ps_q",
                                                name=f"ps_q{do}")
                                for c in range(NCH):
                                    nc.tensor.matmul(
                                        ps_q[:], wq_t[c][:, do * 128:(do + 1) * 128],
                                        xn[c][:], start=(c == 0), stop=(c == NCH - 1))
                                nc.scalar.mul(QT[do][:], ps_q[:], 1.0 / np.sqrt(DK))
                        # V projection: token-major tiles with ones columns
                        for tt in range(4):
                            gt = qt * 4 + tt
                            ps_v = PS1.tile([128, D], F32, tag="ps_v", bufs=2,
                                            name=f"ps_v{gt}")
                            tsl = slice(tt * 128, (tt + 1) * 128)
                            for c in range(NCH):
                                nc.tensor.matmul(
                                    ps_v[:, 0:512], xn[c][:, tsl],
                                    wv_t[c][:, 0:512],
                                    start=(c == 0), stop=(c == NCH - 1))
                                nc.tensor.matmul(
                                    ps_v[:, 512:768], xn[c][:, tsl],
                                    wv_t[c][:, 512:768],
                                    start=(c == 0), stop=(c == NCH - 1))
                            nc.vector.memset(VA[gt][:], 1.0)
                            nc.vector.tensor_copy(
                                VA[gt][:].rearrange("p (h e) -> p h e",
                                                    e=DK + 1)[:, :, 0:DK],
                                ps_v[:].rearrange("p (h d) -> p h d", d=DK))
                    nc.sync.dma_start(maskT_t[:], maskT)
                    warm_burst(0, PS1, "ps_q", P1, warm_rhs[:], 1, n=16)

                # ---------------- stage 2: attention ------------------------
                with (
                    tc.tile_pool(name="s2", bufs=1) as P2,
                    tc.tile_pool(name="ps2", bufs=1, space="PSUM") as PS2,
                ):
                    warm_burst(1, PS2, "ps_sc", P2, warm_rhs[:], 3, n=12)
                    wo_t = [P2.tile([DK, D], BF16, name=f"wo{h}")
                            for h in range(H)]
                    for h in range(H):
                        nc.sync.dma_start(wo_t[h][:], woT[h])
                    attnU = [P2.tile([DK + 1, T], BF16, name=f"attnU{h}")
                             for h in range(H)]
                    for pc in range(H // 2):
                        heads = (2 * pc, 2 * pc + 1)
                        accs = {h: PS2.tile([128, T], F32, tag="acc", bufs=2,
                                            name=f"acc{h}") for h in heads}
                        probs = {}
                        # phase A: scores + exp + mask, even/odd interleaved
                        for g in range(8):
                            for h in heads:
                                r0 = (h % 2) * DK
                                ps_sc = PS2.tile([128, 1024], F32, tag="ps_sc",
                                                 bufs=3, name=f"ps_sc{h}_{g}")
                                for j in range(2):
                                    kt = 2 * g + j
                                    nc.tensor.matmul(
                                        ps_sc[:, j * T:(j + 1) * T],
                                        KT[pc][r0:r0 + DK, kt * 128:(kt + 1) * 128],
                                        QT[pc][r0:r0 + DK, :],
                                        start=True, stop=True)
                                pr = P2.tile([128, 1024], BF16, tag="probs",
                                             bufs=15, name=f"probs{h}_{g}")
                                nc.scalar.activation(pr[:], ps_sc[:], AF.Exp)
                                nc.vector.tensor_mul(
                                    pr[:], pr[:],
                                    maskT_t[:, g * 1024:(g + 1) * 1024])
                                probs[(h, g)] = pr
                        # phase B: attention @ V (full 128-contract, one mode)
                        for h in heads:
                            for g in range(8):
                                for j in range(2):
                                    kt = 2 * g + j
                                    nc.tensor.matmul(
                                        accs[h][0:DK + 1, :],
                                        VA[kt][:, h * (DK + 1):(h + 1) * (DK + 1)],
                                        probs[(h, g)][:, j * T:(j + 1) * T],
                                        start=(g == 0 and j == 0),
                                        stop=(g == 7 and j == 1))
                        # evacuate raw accumulator (values + sums row)
                        for h in heads:
                            nc.vector.tensor_copy(attnU[h][:],
                                                  accs[h][0:DK + 1, :])
                        # keep-alive: prevent HAM idle-flip at pair boundary
                        warm_burst(4, PS2, "ps_sc", P2, warm_rhs[:], 3, n=4,
                                   dve=True)

                    # batched normalize: Ln run then Exp run, gpsimd bcast
                    lnrows = []
                    for h in range(H):
                        lnrow = P2.tile([1, T], F32, tag="lnrow", bufs=2,
                                        name=f"lnrow{h}")
                        nc.scalar.activation(lnrow[:], attnU[h][DK:DK + 1, :],
                                             AF.Ln)
                        lnrows.append(lnrow)
                    for h in range(H):
                        srow = P2.tile([1, T], F32, tag="srow", bufs=2,
                                       name=f"srow{h}")
                        nc.scalar.activation(srow[:], lnrows[h][:], AF.Exp,
                                             scale=-1.0)
                        bc_sb = P2.tile([DK, T], F32, tag="bc_sb", bufs=2,
                                        name=f"bc_sb{h}")
                        nc.gpsimd.partition_broadcast(bc_sb[:], srow[:])
                        nc.vector.tensor_mul(attnT[h][:], attnU[h][0:DK, :],
                                             bc_sb[:])
                    # wo projection + residual in the warm stage-2 stream
                    for do in range(NCH):
                        ps_h2 = PS2.tile([128, T], F32, tag="acc", bufs=2,
                                         name=f"ps_h2_{do}")
                        for h in range(H):
                            nc.tensor.matmul(
                                ps_h2[:], wo_t[h][:, do * 128:(do + 1) * 128],
                                attnT[h][:], start=(h == 0), stop=(h == H - 1))
                        nc.vector.tensor_add(hT[do][:], ps_h2[:], xloc[do][:])
                    warm_burst(4, PS2, "ps_sc", P2, warm_rhs[:], 3, n=30)
                    warm_burst(4, PS2, "ps_sc", P2, warm_rhs[:], 3, n=30)

            # ------------- stage 3+4: wo + residual + FFN (one scope) -------
            with (
                tc.tile_pool(name="s4", bufs=1) as P4,
                tc.tile_pool(name="ps4", bufs=1, space="PSUM") as PS4,
            ):
                warm_burst(2, PS4, "ps_h", P4, warm_rhs[:], 2, n=16)
                ps_ms2 = PS4.tile([128, T], F32, tag="ps_ms2", name="ps_ms2")
                for do in range(NCH):
                    sqh = P4.tile([128, T], BF16, tag="sqh", bufs=2,
                                  name=f"sqh{do}")
                    nc.scalar.activation(sqh[:], hT[do][:], AF.Square)
                    nc.tensor.matmul(ps_ms2[:], ones16_t[:], sqh[:],
                                     start=(do == 0), stop=(do == NCH - 1))
                lntmp2 = P4.tile([128, T], F32, name="lntmp2")
                nc.scalar.activation(lntmp2[:], ps_ms2[:], AF.Ln,
                                     bias=eps_t[:], scale=1.0 / D)
                rstd2 = P4.tile([128, T], F32, name="rstd2")
                nc.scalar.activation(rstd2[:], lntmp2[:], AF.Exp, scale=-0.5)
                hnT = [P4.tile([128, T], BF16, name=f"hnT{c}") for c in range(NCH)]
                for c in range(NCH):
                    nc.vector.tensor_mul(hnT[c][:], hT[c][:], rstd2[:])

                warm_burst(3, PS4, "ps_u", P4, warm_rhs[:], 2, n=16)
                prod = [P4.tile([128, T], BF16, name=f"prod{f}") for f in range(NFC)]
                for f in range(NFC):
                    w1_t = Pg.tile([128, D], BF16, tag="w1_t", bufs=3,
                                   name=f"w1_{f}")
                    nc.sync.dma_start(w1_t[:], w1T[f])
                    w3_t = Pg.tile([128, D], BF16, tag="w3_t", bufs=3,
                                   name=f"w3_{f}")
                    nc.sync.dma_start(w3_t[:], w3T[f])
                    ps_u = PS4.tile([128, T], F32, tag="ps_u", bufs=2,
                                    name=f"ps_u{f}")
                    ps_w = PS4.tile([128, T], F32, tag="ps_w", bufs=2,
                                    name=f"ps_w{f}")
                    for c in range(NCH):
                        csl = slice(c * 128, (c + 1) * 128)
                        nc.tensor.matmul(ps_u[:], w1_t[:, csl], hnT[c][:],
                                         start=(c == 0), stop=(c == NCH - 1))
                        nc.tensor.matmul(ps_w[:], w3_t[:, csl], hnT[c][:],
                                         start=(c == 0), stop=(c == NCH - 1))
                    silu = P4.tile([128, T], BF16, tag="silu", bufs=2,
                                   name=f"silu{f}")
                    if os.environ.get("BASS_SIM_SILU") == "1":
                        # CoreSim has no Silu; emulate as u*sigmoid(u)
                        nc.scalar.activation(silu[:], ps_u[:], AF.Sigmoid)
                        nc.vector.tensor_mul(silu[:], silu[:], ps_u[:])
                    else:
                        nc.scalar.activation(silu[:], ps_u[:], AF.Silu)
                    nc.vector.tensor_mul(prod[f][:], silu[:], ps_w[:])

                for do in range(NCH):
                    w2_t = Pg.tile([128, F], BF16, tag="w2_t", bufs=2,
                                   name=f"w2_{do}")
                    nc.sync.dma_start(w2_t[:], w2T[do])
                    ps_y = PS4.tile([128, T], F32, tag="ps_h", bufs=2,
                                    name=f"ps_y{do}")
                    for f in range(NFC):
                        fsl = slice(f * 128, (f + 1) * 128)
                        nc.tensor.matmul(ps_y[:], w2_t[:, fsl], prod[f][:],
                                         start=(f == 0), stop=(f == NFC - 1))
                    outt = P4.tile([128, T], F32, tag="outt", bufs=2,
                                   name=f"outt{do}")
                    nc.vector.tensor_add(outt[:], ps_y[:], hT[do][:])
                    nc.sync.dma_start(outT[do], outt[:])

    nc.compile()
    return nc


# QT_IDX is the local query-slice index within the batch element. The program
# references it at build time; all cores share one NEFF, so it must be a
# compile-time constant -- we build one NEFF per slice index would be wasteful.
# Instead we make the program identical across cores by noting that the only
# per-core difference stage 1 uses is WHICH quarter is local. To keep a single
# NEFF, the host rotates the token axis per core so that the local slice is
# always quarter 0 (see kernel()).
QT_IDX = 0


def prep_inputs(x, mask, wq, wk, wv, wo, w1, w2, w3, g_attn, g_ffn):
    """Build the 8 per-core input maps (host-side sharding + layout)."""
    bf = ml_dtypes.bfloat16
    wqTe = np.ascontiguousarray((wq * g_attn[None, :]).T.reshape(NCH, 128, D)).astype(bf)
    wkTe = np.ascontiguousarray((wk * g_attn[None, :]).T.reshape(NCH, 128, D)).astype(bf)
    wvTe = np.ascontiguousarray((wv * g_attn[None, :]).T.reshape(NCH, 128, D)).astype(bf)
    woTe = np.ascontiguousarray(wo.T.reshape(H, DK, D)).astype(bf)
    w1Te = np.ascontiguousarray(
        (w1 * g_ffn[None, :]).T.reshape(NCH, 128, NFC, 128)
        .transpose(2, 1, 0, 3).reshape(NFC, 128, D)).astype(bf)
    w3Te = np.ascontiguousarray(
        (w3 * g_ffn[None, :]).T.reshape(NCH, 128, NFC, 128)
        .transpose(2, 1, 0, 3).reshape(NFC, 128, D)).astype(bf)
    w2Te = np.ascontiguousarray(
        w2.T.reshape(NFC, 128, NCH, 128).transpose(2, 1, 0, 3)
        .reshape(NCH, 128, F)).astype(bf)
    ones16 = np.ones((128, 128), bf)

    in_maps = []
    for core in range(8):
        b, qt = core // NQT, core % NQT
        # rotate tokens so the local 512-query slice is always quarter 0
        order = (np.arange(S) + qt * T) % S
        xb = x[b][order]                       # [S, D] rotated
        xTe = np.ascontiguousarray(xb.T.reshape(NCH, 128, S)).astype(np.float32)
        # maskT[p, kt*T + q] = mask[b, qt*T + q, k] with k = kt*128 + p in
        # ROTATED key order (keys follow the same rotation as tokens).
        msl = mask[b, qt * T:(qt + 1) * T][:, order]     # [T(q), S(k)] rotated
        maskTe = np.ascontiguousarray(
            msl.T.reshape(NKT, 128, T).transpose(1, 0, 2)
            .reshape(128, NKT * T)).astype(bf)
        in_maps.append({
            "xT": xTe, "maskT": maskTe,
            "wqT": wqTe, "wkT": wkTe, "wvT": wvTe, "woT": woTe,
            "w1T": w1Te, "w3T": w3Te, "w2T": w2Te,
            "ones16": ones16,
        })
    return in_maps


_NC_CACHE = None


def get_nc():
    global _NC_CACHE
    if _NC_CACHE is None:
        _NC_CACHE = build_nc()
    return _NC_CACHE


def gather_output(results):
    out = np.empty((B, S, D), np.float32)
    for core in range(8):
        b, qt = core // NQT, core % NQT
        o = results[core]["outT"]              # [NCH, 128, T]
        out[b, qt * T:(qt + 1) * T, :] = o.reshape(D, T).T
    return out


def kernel(**inputs):
    from concourse.bass_utils import run_bass_kernel_spmd
    in_maps = prep_inputs(
        np.asarray(inputs["x"]), np.asarray(inputs["mask"]),
        np.asarray(inputs["wq"]), np.asarray(inputs["wk"]),
        np.asarray(inputs["wv"]), np.asarray(inputs["wo"]),
        np.asarray(inputs["w1"]), np.asarray(inputs["w2"]),
        np.asarray(inputs["w3"]),
        np.asarray(inputs["g_attn"]), np.asarray(inputs["g_ffn"]))
    nc = get_nc()
    res = run_bass_kernel_spmd(nc, in_maps, core_ids=list(range(8)))
    return gather_output(res.results)



# revision 4
# speedup vs baseline: 1.1600x; 1.1600x over previous
"""Trainium2 Bass kernel for a pre-norm transformer encoder layer (SwiGLU FFN).

Shapes (hardcoded): x [2, 2048, 768], mask [2, 2048, 2048] int32,
wq/wk/wv/wo [768, 768], w1/w3 [3072, 768], w2 [768, 3072], g_attn/g_ffn [768].

Sharding: 8 cores = 2 batch x 4 query-slices of 512 tokens. Each core
computes K/V for its full batch element (replicated within the group of 4)
and attention + FFN for its own 512 tokens. No collectives.

On-device layout is feature-major ("transposed"): activations [D, tokens].
QKV projections and the FFN w1/w3 matmuls run in fp8e4 DoubleRow (2x PE);
scores / attnV / wo / w2 run in bf16. All accumulation fp32 in PSUM.
"""
import os
import sys

for _p in ("/opt/trn_rl_repo", "/root/.axon_site/_ro/trn_rl_repo"):
    if os.path.isdir(_p) and _p not in sys.path:
        sys.path.append(_p)

import numpy as np
import ml_dtypes

import concourse.bacc as bacc
import concourse.tile as tile
from concourse import mybir

F32 = mybir.dt.float32
BF16 = mybir.dt.bfloat16
F8 = mybir.dt.float8e4
AF = mybir.ActivationFunctionType
DR = mybir.MatmulPerfMode.DoubleRow

B, S, D, H = 2, 2048, 768, 12
DK = D // H            # 64
F = 4 * D              # 3072
T = 512                # local query tokens per core
NCH = D // 128         # 6 feature chunks
NCP = NCH // 2         # 3 feature chunk-pairs (fp8 DoubleRow)
NFC = F // 128         # 24 FFN chunks
NKT = S // 128         # 16 key tiles
NQT = S // T           # 4 query slices per batch element
EPS = 1e-5
RD = 1.0 / D

# act_info.json table-set ids (see hw_specs.get_activation_tables):
#   6 = natural_log_exp_and_others (exp, ln, copy, square, identity)
ACT_SET_LNEXP = 6
W13_BUFS = 8           # fp8 FFN weight prefetch ring depth


def dr3(ap):
    """View a [128, 2*N] AP as the DoubleRow 3D form [128, 2, N]."""
    return ap.rearrange("p (j n) -> p j n", j=2)


def build_nc():
    nc = bacc.Bacc("TRN2", target_bir_lowering=False, debug=False, num_devices=8)

    xT = nc.dram_tensor("xT", [NCH, 128, S], F32, kind="ExternalInput").ap()
    maskT = nc.dram_tensor("maskT", [128, NKT * T], BF16, kind="ExternalInput").ap()
    # fp8 DoubleRow weights: [cp, p, (j, m)] with d = cp*256 + j*128 + p
    wq8 = nc.dram_tensor("wq8", [NCP, 128, 2 * D], F8, kind="ExternalInput").ap()
    wk8 = nc.dram_tensor("wk8", [NCP, 128, 2 * D], F8, kind="ExternalInput").ap()
    wv8 = nc.dram_tensor("wv8", [NCP, 128, 2 * D], F8, kind="ExternalInput").ap()
    woT = nc.dram_tensor("woT", [H, DK, D], BF16, kind="ExternalInput").ap()
    # [f, p, (cp, j, m)]
    w1_8 = nc.dram_tensor("w1_8", [NFC, 128, D], F8, kind="ExternalInput").ap()
    w3_8 = nc.dram_tensor("w3_8", [NFC, 128, D], F8, kind="ExternalInput").ap()
    w2T = nc.dram_tensor("w2T", [NCH, 128, F], BF16, kind="ExternalInput").ap()
    ones8 = nc.dram_tensor("ones8", [128, 256], F8, kind="ExternalInput").ap()
    ones16 = nc.dram_tensor("ones16", [128, 128], BF16, kind="ExternalInput").ap()

    outT = nc.dram_tensor("outT", [NCH, 128, T], F32, kind="ExternalOutput").ap()
    warm_out = nc.dram_tensor("warm_out", [2, 128, T], BF16,
                              kind="ExternalOutput").ap()

    with tile.TileContext(nc) as tc:
        with tc.tile_pool(name="glob", bufs=1) as Pg:
            # pin the exp+ln activation table once; silu triggers one switch
            nc.scalar.add_instruction(mybir.InstLoadActFuncSet(
                name=nc.get_next_instruction_name(), ins=[], outs=[],
                act_func_set_id=ACT_SET_LNEXP))

            ones16_t = Pg.tile([128, 128], BF16, name="ones16_t")
            nc.sync.dma_start(ones16_t[:], ones16)
            ones8_t = Pg.tile([128, 256], F8, name="ones8_t")
            nc.sync.dma_start(ones8_t[:], ones8)

            def warm_burst(idx, psum_pool, tag, sbuf_pool, rhs, pbufs, n=12):
                """Dense matmul burst to trip the PE HAM to 2.4GHz."""
                wp = psum_pool.tile([128, T], F32, tag=tag, bufs=pbufs,
                                    name=f"wrm_ps{idx}")
                for i in range(n):
                    nc.tensor.matmul(wp[:], ones16_t[:], rhs,
                                     start=(i == 0), stop=(i == n - 1))
                ws = sbuf_pool.tile([128, T], BF16, tag="wrm_sb",
                                    name=f"wrm_sb{idx}")
                nc.scalar.copy(ws[:], wp[:])
                nc.sync.dma_start(warm_out[idx], ws[:])

            eps_t = Pg.tile([128, 1], F32, name="eps_t")
            nc.vector.memset(eps_t[:], EPS)
            # xloc holds x (residual) during attention, then h in-place
            xloc = [Pg.tile([128, T], F32, name=f"xloc{c}") for c in range(NCH)]
            warm_rhs = Pg.tile([128, T], BF16, name="warm_rhs")

            # fp8 FFN weight prefetch rings
            w1r, w3r = {}, {}

            def w13_fetch(f):
                w1r[f] = Pg.tile([128, D], F8, tag="w1r", bufs=W13_BUFS,
                                 name=f"w1_{f}")
                nc.sync.dma_start(w1r[f][:], w1_8[f])
                w3r[f] = Pg.tile([128, D], F8, tag="w3r", bufs=W13_BUFS,
                                 name=f"w3_{f}")
                nc.sync.dma_start(w3r[f][:], w3_8[f])

            with tc.tile_pool(name="attn", bufs=1) as Pa:
                KT = [Pa.tile([128, S], BF16, name=f"KT{c}") for c in range(NCH)]
                QT = [Pa.tile([128, T], BF16, name=f"QT{c}") for c in range(NCH)]
                VA = [Pa.tile([128, H * (DK + 1)], BF16, name=f"VA{t}")
                      for t in range(NKT)]
                maskT_t = Pa.tile([128, NKT * T], BF16, name="maskT_t")

                # ---------------- stage 1: rmsnorm + Q/K/V projections --------
                with (
                    tc.tile_pool(name="s1", bufs=1) as P1,
                    tc.tile_pool(name="ps1", bufs=1, space="PSUM") as PS1,
                ):
                    wq_t = [P1.tile([128, 2 * D], F8, name=f"wq{c}")
                            for c in range(NCP)]
                    wk_t = [P1.tile([128, 2 * D], F8, name=f"wk{c}")
                            for c in range(NCP)]
                    wv_t = [P1.tile([128, 2 * D], F8, name=f"wv{c}")
                            for c in range(NCP)]

                    # prefetch first x slices + weights, warm the PE meanwhile
                    xq_tiles = {}
                    for qt in range(2):
                        xq_tiles[qt] = [P1.tile([128, T], F32, name=f"xq{qt}_{c}",
                                                tag=f"xq{c}", bufs=2)
                                        for c in range(NCH)]
                        for c in range(NCH):
                            nc.sync.dma_start(xq_tiles[qt][c][:],
                                              xT[c][:, qt * T:(qt + 1) * T])
                    for c in range(NCP):
                        nc.sync.dma_start(wq_t[c][:], wq8[c])
                        nc.sync.dma_start(wk_t[c][:], wk8[c])
                        nc.sync.dma_start(wv_t[c][:], wv8[c])
                    nc.sync.dma_start(warm_rhs[:], maskT[:, 0:T])
                    nc.sync.dma_start(maskT_t[:], maskT)
                    warm_burst(0, PS1, "ps_k", P1, warm_rhs[:], 2, n=20)

                    def rms_head(qt):
                        """Square + mean-square matmul for slice qt."""
                        xq = xq_tiles[qt]
                        sq8 = [P1.tile([128, 2 * T], F8, name=f"sq{qt}_{p}",
                                       tag=f"sq{p}", bufs=2) for p in range(NCP)]
                        for c in range(NCH):
                            nc.scalar.activation(
                                sq8[c // 2][:, (c % 2) * T:(c % 2 + 1) * T],
                                xq[c][:], AF.Square)
                        ps_ms = PS1.tile([128, T], F32, tag="ps_ms", bufs=2,
                                         name=f"ps_ms{qt}")
                        for p in range(NCP):
                            nc.tensor.matmul(ps_ms[:], dr3(ones8_t[:]),
                                             dr3(sq8[p][:]), start=(p == 0),
                                             stop=(p == NCP - 1), perf_mode=DR)
                        return ps_ms

                    def rms_tail(qt, ps_ms):
                        """rstd + fp8 normalized activations for slice qt."""
                        xq = xq_tiles[qt]
                        lntmp = P1.tile([128, T], F32, tag="lntmp", bufs=2,
                                        name=f"ln{qt}")
                        nc.scalar.activation(lntmp[:], ps_ms[:], AF.Ln,
                                             bias=eps_t[:], scale=RD)
                        rstd = P1.tile([128, T], F32, tag="rstd", bufs=2,
                                       name=f"rstd{qt}")
                        nc.scalar.activation(rstd[:], lntmp[:], AF.Exp,
                                             scale=-0.5)
                        xn8 = [P1.tile([128, 2 * T], F8, name=f"xn{qt}_{p}",
                                       tag=f"xn{p}", bufs=2) for p in range(NCP)]
                        for c in range(NCH):
                            nc.vector.tensor_mul(
                                xn8[c // 2][:, (c % 2) * T:(c % 2 + 1) * T],
                                xq[c][:], rstd[:])
                        return xn8

                    ps_ms = rms_head(0)
                    xn8 = rms_tail(0, ps_ms)

                    for qt in range(NQT):
                        sl = slice(qt * T, (qt + 1) * T)
                        local = (qt == 0)
                        if local:
                            for c in range(NCH):
                                nc.vector.tensor_copy(xloc[c][:],
                                                      xq_tiles[0][c][:])
                        # K (and local Q) projections: fp8 DoubleRow
                        for do in range(NCH):
                            dsl = slice(do * 128, (do + 1) * 128)
                            ps_k = PS1.tile([128, T], F32, tag="ps_k", bufs=2,
                                            name=f"ps_k{qt}_{do}")
                            for p in range(NCP):
                                nc.tensor.matmul(
                                    ps_k[:], dr3(wk_t[p][:])[:, :, dsl],
                                    dr3(xn8[p][:]), start=(p == 0),
                                    stop=(p == NCP - 1), perf_mode=DR)
                            nc.scalar.copy(KT[do][:, sl], ps_k[:])
                            if local:
                                ps_q = PS1.tile([128, T], F32, tag="ps_k",
                                                bufs=2, name=f"ps_q{do}")
                                for p in range(NCP):
                                    nc.tensor.matmul(
                                        ps_q[:], dr3(wq_t[p][:])[:, :, dsl],
                                        dr3(xn8[p][:]), start=(p == 0),
                                        stop=(p == NCP - 1), perf_mode=DR)
                                nc.scalar.mul(QT[do][:], ps_q[:],
                                              1.0 / np.sqrt(DK))
                        # prefetch + rmsnorm head for the next slice while the
                        # V projections keep the PE busy
                        nxt = qt + 1
                        if nxt < NQT:
                            if nxt + 1 < NQT:
                                xq_tiles[nxt + 1] = [
                                    P1.tile([128, T], F32, tag=f"xq{c}", bufs=2,
                                            name=f"xq{nxt + 1}_{c}")
                                    for c in range(NCH)]
                                for c in range(NCH):
                                    nc.sync.dma_start(
                                        xq_tiles[nxt + 1][c][:],
                                        xT[c][:, (nxt + 1) * T:(nxt + 2) * T])
                            ps_ms_n = rms_head(nxt)
                        # V projection: token-major via xn8 as stationary
                        for tt in range(4):
                            gt = qt * 4 + tt
                            tsl = slice(tt * 128, (tt + 1) * 128)
                            ps_v = PS1.tile([128, D], F32, tag="ps_v", bufs=2,
                                            name=f"ps_v{gt}")
                            for p in range(NCP):
                                nc.tensor.matmul(
                                    ps_v[:, 0:512], dr3(xn8[p][:])[:, :, tsl],
                                    dr3(wv_t[p][:])[:, :, 0:512],
                                    start=(p == 0), stop=(p == NCP - 1),
                                    perf_mode=DR)
                                nc.tensor.matmul(
                                    ps_v[:, 512:768], dr3(xn8[p][:])[:, :, tsl],
                                    dr3(wv_t[p][:])[:, :, 512:768],
                                    start=(p == 0), stop=(p == NCP - 1),
                                    perf_mode=DR)
                            nc.gpsimd.memset(VA[gt][:], 1.0)
                            nc.vector.tensor_copy(
                                VA[gt][:].rearrange("p (h e) -> p h e",
                                                    e=DK + 1)[:, :, 0:DK],
                                ps_v[:].rearrange("p (h d) -> p h d", d=DK))
                        if nxt < NQT:
                            xn8 = rms_tail(nxt, ps_ms_n)
                    for f in range(W13_BUFS):
                        w13_fetch(f)

                # ---------------- stage 2: attention ------------------------
                with (
                    tc.tile_pool(name="s2", bufs=1) as P2,
                    tc.tile_pool(name="ps2", bufs=1, space="PSUM") as PS2,
                ):
                    wo_t = [P2.tile([DK, D], BF16, name=f"wo{h}")
                            for h in range(H)]
                    for h in range(H):
                        nc.sync.dma_start(wo_t[h][:], woT[h])
                    attnT = [P2.tile([DK, T], BF16, name=f"attnT{h}")
                             for h in range(H)]
                    srows = P2.tile([1, 2 * T], F32, name="srows")

                    def phase_a(pc, g, probs):
                        """Scores + exp + mask for head pair pc, group g."""
                        heads = (2 * pc, 2 * pc + 1)
                        pss = {}
                        for j in range(2):          # interleave row groups
                            kt = 2 * g + j
                            ksl = slice(kt * 128, (kt + 1) * 128)
                            for h in heads:
                                r0 = (h % 2) * DK
                                if j == 0:
                                    pss[h] = PS2.tile([128, 1024], F32,
                                                      tag="ps_sc", bufs=2,
                                                      name=f"ps_sc{h}_{g}")
                                nc.tensor.matmul(
                                    pss[h][:, j * T:(j + 1) * T],
                                    KT[pc][r0:r0 + DK, ksl],
                                    QT[pc][r0:r0 + DK, :],
                                    start=True, stop=True)
                        for h in heads:
                            pr = P2.tile([128, 1024], BF16, tag="probs",
                                         bufs=6, name=f"probs{h}_{g}")
                            nc.scalar.activation(pr[:], pss[h][:], AF.Exp)
                            nc.vector.tensor_mul(
                                pr[:], pr[:],
                                maskT_t[:, g * 1024:(g + 1) * 1024])
                            probs[(h, g)] = pr

                    def phase_b(pc, g, accs, probs):
                        """attn @ V accumulation for head pair pc, group g."""
                        for h in (2 * pc, 2 * pc + 1):
                            pr = probs.pop((h, g))
                            for j in range(2):
                                kt = 2 * g + j
                                nc.tensor.matmul(
                                    accs[h][0:DK + 1, :],
                                    VA[kt][:, h * (DK + 1):(h + 1) * (DK + 1)],
                                    pr[:, j * T:(j + 1) * T],
                                    start=(g == 0 and j == 0),
                                    stop=(g == 7 and j == 1))

                    def pair_tail(pc, accs):
                        """1/sum + broadcast + normalized attnT for pair pc."""
                        heads = (2 * pc, 2 * pc + 1)
                        for i, h in enumerate(heads):
                            nc.vector.tensor_copy(
                                srows[0:1, i * T:(i + 1) * T],
                                accs[h][DK:DK + 1, :])
                        lnr = P2.tile([1, 2 * T], F32, tag="lnr", bufs=2,
                                      name=f"lnr{pc}")
                        nc.scalar.activation(lnr[:], srows[:], AF.Ln)
                        srec = P2.tile([1, 2 * T], F32, tag="srec", bufs=2,
                                       name=f"srec{pc}")
                        nc.scalar.activation(srec[:], lnr[:], AF.Exp,
                                             scale=-1.0)
                        for i, h in enumerate(heads):
                            bc = P2.tile([DK, T], F32, tag="bc", bufs=2,
                                         name=f"bc{h}")
                            nc.gpsimd.partition_broadcast(
                                bc[:], srec[0:1, i * T:(i + 1) * T])
                            nc.vector.tensor_mul(attnT[h][:],
                                                 accs[h][0:DK, :], bc[:])

                    accs_by_pc = {}
                    probs = {}
                    prev_tail = None
                    for pc in range(H // 2):
                        heads = (2 * pc, 2 * pc + 1)
                        accs = {h: PS2.tile([128, T], F32, tag="acc", bufs=4,
                                            name=f"acc{h}") for h in heads}
                        accs_by_pc[pc] = accs
                        # lockstep: A(g) then B(g-1); B trails by one group
                        for g in range(8):
                            phase_a(pc, g, probs)
                            if g >= 1:
                                phase_b(pc, g - 1, accs, probs)
                            if g == 2 and prev_tail is not None:
                                prev_tail()
                        phase_b(pc, 7, accs, probs)
                        prev_tail = (lambda p=pc: pair_tail(p, accs_by_pc[p]))
                    prev_tail()

                    # wo projection + residual (in place into xloc -> h)
                    for do in range(NCH):
                        ps_h2 = PS2.tile([128, T], F32, tag="ps_sc", bufs=2,
                                         name=f"ps_h2_{do}")
                        for h in range(H):
                            nc.tensor.matmul(
                                ps_h2[:], wo_t[h][:, do * 128:(do + 1) * 128],
                                attnT[h][:], start=(h == 0), stop=(h == H - 1))
                        nc.vector.tensor_add(xloc[do][:], ps_h2[:], xloc[do][:])

            # ------------- stage 3+4: FFN (SwiGLU) --------------------------
            with (
                tc.tile_pool(name="s4", bufs=1) as P4,
                tc.tile_pool(name="ps4", bufs=1, space="PSUM") as PS4,
            ):
                # rmsnorm on h (in xloc) -> hn8 (fp8 pair tiles)
                sqh8 = [P4.tile([128, 2 * T], F8, name=f"sqh{p}")
                        for p in range(NCP)]
                for c in range(NCH):
                    nc.scalar.activation(
                        sqh8[c // 2][:, (c % 2) * T:(c % 2 + 1) * T],
                        xloc[c][:], AF.Square)
                ps_ms2 = PS4.tile([128, T], F32, tag="ps_u", bufs=2,
                                  name="ps_ms2")
                for p in range(NCP):
                    nc.tensor.matmul(ps_ms2[:], dr3(ones8_t[:]),
                                     dr3(sqh8[p][:]), start=(p == 0),
                                     stop=(p == NCP - 1), perf_mode=DR)
                lntmp2 = P4.tile([128, T], F32, name="lntmp2")
                nc.scalar.activation(lntmp2[:], ps_ms2[:], AF.Ln,
                                     bias=eps_t[:], scale=RD)
                rstd2 = P4.tile([128, T], F32, name="rstd2")
                nc.scalar.activation(rstd2[:], lntmp2[:], AF.Exp, scale=-0.5)
                hn8 = [P4.tile([128, 2 * T], F8, name=f"hn{p}")
                       for p in range(NCP)]
                for c in range(NCH):
                    nc.vector.tensor_mul(
                        hn8[c // 2][:, (c % 2) * T:(c % 2 + 1) * T],
                        xloc[c][:], rstd2[:])

                w2_t = {}
                for do in range(2):
                    w2_t[do] = P4.tile([128, F], BF16, tag="w2_t", bufs=2,
                                       name=f"w2_{do}")
                    nc.sync.dma_start(w2_t[do][:], w2T[do])

                prod = [P4.tile([128, T], BF16, name=f"prod{f}")
                        for f in range(NFC)]
                for f in range(NFC):
                    if f + W13_BUFS < NFC:
                        w13_fetch(f + W13_BUFS)
                    ps_u = PS4.tile([128, T], F32, tag="ps_u", bufs=2,
                                    name=f"ps_u{f}")
                    ps_w = PS4.tile([128, T], F32, tag="ps_w", bufs=2,
                                    name=f"ps_w{f}")
                    for p in range(NCP):
                        csl = slice(p * 256, (p + 1) * 256)
                        nc.tensor.matmul(
                            ps_u[:], dr3(w1r[f][:, csl]), dr3(hn8[p][:]),
                            start=(p == 0), stop=(p == NCP - 1), perf_mode=DR)
                        nc.tensor.matmul(
                            ps_w[:], dr3(w3r[f][:, csl]), dr3(hn8[p][:]),
                            start=(p == 0), stop=(p == NCP - 1), perf_mode=DR)
                    silu = P4.tile([128, T], BF16, tag="silu", bufs=2,
                                   name=f"silu{f}")
                    if os.environ.get("BASS_SIM_SILU") == "1":
                        # CoreSim has no Silu; emulate as u*sigmoid(u)
                        nc.scalar.activation(silu[:], ps_u[:], AF.Sigmoid)
                        nc.vector.tensor_mul(silu[:], silu[:], ps_u[:])
                    else:
                        nc.scalar.activation(silu[:], ps_u[:], AF.Silu)
                    nc.vector.tensor_mul(prod[f][:], silu[:], ps_w[:])

                for do in range(NCH):
                    if do + 2 < NCH:
                        w2_t[do + 2] = P4.tile([128, F], BF16, tag="w2_t",
                                               bufs=2, name=f"w2_{do + 2}")
                        nc.sync.dma_start(w2_t[do + 2][:], w2T[do + 2])
                    ps_y = PS4.tile([128, T], F32, tag="ps_y", bufs=2,
                                    name=f"ps_y{do}")
                    for f in range(NFC):
                        fsl = slice(f * 128, (f + 1) * 128)
                        nc.tensor.matmul(ps_y[:], w2_t[do][:, fsl], prod[f][:],
                                         start=(f == 0), stop=(f == NFC - 1))
                    outt = P4.tile([128, T], F32, tag="outt", bufs=2,
                                   name=f"outt{do}")
                    nc.vector.tensor_add(outt[:], ps_y[:], xloc[do][:])
                    nc.sync.dma_start(outT[do], outt[:])

    nc.compile()
    return nc


def _f8(a):
    return np.clip(a, -240.0, 240.0).astype(ml_dtypes.float8_e4m3)


def prep_inputs(x, mask, wq, wk, wv, wo, w1, w2, w3, g_attn, g_ffn):
    """Build the 8 per-core input maps (host-side sharding + layout)."""
    bf = ml_dtypes.bfloat16

    def dr_w(w, g):
        # [cp, p, (j, m)]: lhsT[p, j, m] = (w*g)[m, cp*256 + j*128 + p]
        wt = (w * g[None, :]).T                      # [d_in, d_out]
        return _f8(np.ascontiguousarray(
            wt.reshape(NCP, 2, 128, D).transpose(0, 2, 1, 3)
            .reshape(NCP, 128, 2 * D)))

    wq8 = dr_w(wq, g_attn)
    wk8 = dr_w(wk, g_attn)
    wv8 = dr_w(wv, g_attn)
    woTe = np.ascontiguousarray(wo.T.reshape(H, DK, D)).astype(bf)

    def dr_ffn(w, g):
        # [f, p, (cp, j, m)]: lhsT[p, cp, j, m] = (w*g)[f*128+m, cp*256+j*128+p]
        wt = (w * g[None, :]).T                      # [D, F]
        t = wt.reshape(NCP, 2, 128, NFC, 128).transpose(3, 2, 0, 1, 4)
        return _f8(np.ascontiguousarray(t.reshape(NFC, 128, D)))

    w1_8 = dr_ffn(w1, g_ffn)
    w3_8 = dr_ffn(w3, g_ffn)
    w2Te = np.ascontiguousarray(
        w2.T.reshape(NFC, 128, NCH, 128).transpose(2, 1, 0, 3)
        .reshape(NCH, 128, F)).astype(bf)
    ones8 = np.ones((128, 256), ml_dtypes.float8_e4m3)
    ones16 = np.ones((128, 128), bf)

    in_maps = []
    for core in range(8):
        b, qt = core // NQT, core % NQT
        # rotate tokens so the local 512-query slice is always quarter 0
        order = (np.arange(S) + qt * T) % S
        xb = x[b][order]                       # [S, D] rotated
        xTe = np.ascontiguousarray(xb.T.reshape(NCH, 128, S)).astype(np.float32)
        # maskT[p, kt*T + q] = mask[b, qt*T + q, k] with k = kt*128 + p in
        # ROTATED key order (keys follow the same rotation as tokens).
        msl = mask[b, qt * T:(qt + 1) * T][:, order]     # [T(q), S(k)] rotated
        maskTe = np.ascontiguousarray(
            msl.T.reshape(NKT, 128, T).transpose(1, 0, 2)
            .reshape(128, NKT * T)).astype(bf)
        in_maps.append({
            "xT": xTe, "maskT": maskTe,
            "wq8": wq8, "wk8": wk8, "wv8": wv8, "woT": woTe,
            "w1_8": w1_8, "w3_8": w3_8, "w2T": w2Te,
            "ones8": ones8, "ones16": ones16,
        })
    return in_maps


_NC_CACHE = None


def get_nc():
    global _NC_CACHE
    if _NC_CACHE is None:
        _NC_CACHE = build_nc()
    return _NC_CACHE


def gather_output(results):
    out = np.empty((B, S, D), np.float32)
    for core in range(8):
        b, qt = core // NQT, core % NQT
        o = results[core]["outT"]              # [NCH, 128, T]
        out[b, qt * T:(qt + 1) * T, :] = o.reshape(D, T).T
    return out


def kernel(**inputs):
    from concourse.bass_utils import run_bass_kernel_spmd
    in_maps = prep_inputs(
        np.asarray(inputs["x"]), np.asarray(inputs["mask"]),
        np.asarray(inputs["wq"]), np.asarray(inputs["wk"]),
        np.asarray(inputs["wv"]), np.asarray(inputs["wo"]),
        np.asarray(inputs["w1"]), np.asarray(inputs["w2"]),
        np.asarray(inputs["w3"]),
        np.asarray(inputs["g_attn"]), np.asarray(inputs["g_ffn"]))
    nc = get_nc()
    res = run_bass_kernel_spmd(nc, in_maps, core_ids=list(range(8)))
    return gather_output(res.results)
